# revision 9
# baseline (speedup 1.0000x reference)
"""GraphSAGE 2-layer fraud detector on 8 trn2 NeuronCores.

Strategy (dst-partitioned, matmul scatter, minimal host->device traffic):
  - The axon tunnel moves ~45MB/s, so wall time is dominated by (a) input
    bytes shipped per spmd call and (b) per-call re-lowering of the kernel
    BIR (proportional to instruction count). Each core receives ONLY its
    x shard (fp16, 1.6MB) plus compressed edge tables; x is AllGathered
    across cores on-device into DRAM, and everything else (iota, identity,
    x^T blocks) is derived on-device.
  - Nodes padded to 50176 = 8 cores x 49 blocks x 128. Core c owns nodes
    [c*6272, (c+1)*6272). Within a core, dst block b holds the 128 nodes
    with local index p*49 + b (p = row in block), which makes the z tile a
    plain contiguous view of z rows in node order.
  - All per-edge work is driven by gpsimd.dma_gather: one instruction
    gathers a whole block's worth of 256B rows from an HBM table into
    SBUF [128, chunks, 128]. Indices are int16, so x_full is addressed as
    two half-tables (rows [0,25088) and [25088,50176)) and each block's
    edges are partitioned by source half (order within a block is free).
  - Layer 1, per block (hardware For_i loop, ~55 instrs total): gather
    msg rows, build one-hot P[e,d]=(ldst[e]==d) per 128-edge chunk on DVE,
    PSUM-accumulate P.T @ msg = agg, scale by 1/deg, then
    hT = relu(W1l @ aggT + W1r @ xbT + b1) via PE transposes.
  - z = h@W2l.T, o = h@W2r.T + b2 (aggregation commutes with the linear
    map, so layer 2 aggregates 2-wide z, not 256-wide h). z is written
    into a 256B-padded node-ordered table, AllGathered, and layer 2 reuses
    the SAME index tables to gather z and scatter into agg2;
    out = recip*agg2 + o.
"""

import time

import numpy as np

import concourse.bass as bass
import concourse.mybir as mybir
import concourse.tile as tile
from concourse import bacc
from concourse.bass import ds, ts
from concourse.bass_utils import run_bass_kernel_spmd

N = 50000
E = 800000
IN_C = 128
HID = 256
OUT_C = 2
NCORES = 8
P = 128
NB = 49                 # dst blocks per core
ROWS = NB * P           # 6272 rows per core
NP = NCORES * ROWS      # 50176 padded nodes
HALF = NP // 2          # 25088 rows per half-table (int16-addressable)

f32 = mybir.dt.float32
f16 = mybir.dt.float16
i32 = mybir.dt.int32
i16 = mybir.dt.int16
u8 = mybir.dt.uint8


def _wrap16(flat):
    """dma_gather index layout: flat j -> [partition j%16, col j//16]."""
    return np.ascontiguousarray(flat.reshape(-1, 16).T)


def _host_prep(x, edge_index, W1l, b1, W1r, W2l, b2, W2r):
    src = edge_index[0].astype(np.int64)
    dst = edge_index[1].astype(np.int64)
    cnt = np.bincount(dst, minlength=NP)
    recip = (1.0 / np.maximum(cnt, 1)).astype(np.float32)

    # dst sort key in block-layout space: node (core c, local r) sits in
    # block b = r % 49 at row p = r // 49 -> key = c*6272 + b*128 + p.
    c_ = dst // ROWS
    r_ = dst % ROWS
    key = c_ * ROWS + (r_ % NB) * P + (r_ // NB)
    order = np.argsort(key, kind="stable")
    s_src = src[order]
    s_key = key[order]

    block_starts = np.searchsorted(s_key, np.arange(0, NP + P, P))

    # per (core, block): split edges by source half, count chunks
    W0 = 1
    W1 = 1
    parts = {}
    for bb in range(NCORES * NB):
        s, e = int(block_starts[bb]), int(block_starts[bb + 1])
        bs = s_src[s:e]
        bl = (s_key[s:e] % P).astype(np.uint8)
        m0 = bs < HALF
        p0s, p0l = bs[m0], bl[m0]
        p1s, p1l = bs[~m0] - HALF, bl[~m0]
        parts[bb] = (p0s, p0l, p1s, p1l)
        W0 = max(W0, -(-len(p0s) // P))
        W1 = max(W1, -(-len(p1s) // P))
    W2 = W0 + W1
    C1 = NB * W2

    idx_arr = np.zeros((NCORES, 16, NB * 8 * W2), np.int16)
    ldst_arr = np.full((NCORES, P, C1), 255, dtype=np.uint8)
    for c in range(NCORES):
        for b in range(NB):
            p0s, p0l, p1s, p1l = parts[c * NB + b]
            i0 = np.zeros(W0 * P, np.int16)
            i0[:len(p0s)] = p0s
            i1 = np.zeros(W1 * P, np.int16)
            i1[:len(p1s)] = p1s
            col = b * 8 * W2
            idx_arr[c, :, col:col + 8 * W0] = _wrap16(i0)
            idx_arr[c, :, col + 8 * W0:col + 8 * W2] = _wrap16(i1)
            l0 = np.full(W0 * P, 255, np.uint8)
            l0[:len(p0l)] = p0l
            l1 = np.full(W1 * P, 255, np.uint8)
            l1[:len(p1l)] = p1l
            ldst_arr[c, :, b * W2:b * W2 + W0] = l0.reshape(W0, P).T
            ldst_arr[c, :, b * W2 + W0:(b + 1) * W2] = l1.reshape(W1, P).T

    x_pad = np.zeros((NP, IN_C), np.float16)
    x_pad[:N] = x.astype(np.float16)
    W1lT = np.ascontiguousarray(W1l.T.astype(np.float16))   # [128, 256]
    W1rT = np.ascontiguousarray(W1r.T.astype(np.float16))
    Wzo = np.zeros((P, 8), np.float16)
    for j in range(2):
        Wzo[:, 4 * j:4 * j + 2] = W2l.T[j * P:(j + 1) * P, :].astype(np.float16)
        Wzo[:, 4 * j + 2:4 * j + 4] = W2r.T[j * P:(j + 1) * P, :].astype(np.float16)
    b1p = np.ascontiguousarray(np.asarray(b1).reshape(2, P).T.astype(np.float32))
    b2b = np.tile(np.asarray(b2).reshape(1, 2), (P, 1)).astype(np.float32)
    recip_c = recip.reshape(NCORES, P, NB).copy()  # node local r = p*49+b

    in_maps = []
    for c in range(NCORES):
        in_maps.append({
            "x_sh": np.ascontiguousarray(x_pad[c * ROWS:(c + 1) * ROWS, :]),
            "idx16": np.ascontiguousarray(idx_arr[c]),
            "ldstu": np.ascontiguousarray(ldst_arr[c]),
            "W1lT": W1lT,
            "W1rT": W1rT,
            "Wzo": Wzo,
            "b1p": b1p,
            "b2b": b2b,
            "recip": np.ascontiguousarray(recip_c[c]),
        })
    return in_maps, W0, W1


def _build(W0, W1):
    W2 = W0 + W1
    C1 = NB * W2
    nc = bacc.Bacc(None, target_bir_lowering=False, debug=False)

    x_sh_d = nc.dram_tensor("x_sh", [ROWS, IN_C], f16, kind="ExternalInput")
    idx_d = nc.dram_tensor("idx16", [16, NB * 8 * W2], i16, kind="ExternalInput")
    ldstu_d = nc.dram_tensor("ldstu", [P, C1], u8, kind="ExternalInput")
    W1lT_d = nc.dram_tensor("W1lT", [P, HID], f16, kind="ExternalInput")
    W1rT_d = nc.dram_tensor("W1rT", [P, HID], f16, kind="ExternalInput")
    Wzo_d = nc.dram_tensor("Wzo", [P, 8], f16, kind="ExternalInput")
    b1p_d = nc.dram_tensor("b1p", [P, 2], f32, kind="ExternalInput")
    b2b_d = nc.dram_tensor("b2b", [P, 2], f32, kind="ExternalInput")
    recip_d = nc.dram_tensor("recip", [P, NB], f32, kind="ExternalInput")
    out_d = nc.dram_tensor("out", [P, 2 * NB], f32, kind="ExternalOutput")

    with tile.TileContext(nc) as tc:
        with (
            tc.tile_pool(name="big", bufs=1) as big,
            tc.tile_pool(name="lp", bufs=4) as lp,
            tc.tile_pool(name="pp", bufs=2, space="PSUM") as pp,
            tc.tile_pool(name="dram", bufs=1, space="DRAM") as dp,
        ):
            def load(d, shape, dt, tag):
                t = big.tile(shape, dt, tag=tag, name=tag)
                nc.sync.dma_start(out=t[:], in_=d[:, :])
                return t

            W1lT_sb = load(W1lT_d, [P, HID], f16, "w1l")
            W1rT_sb = load(W1rT_d, [P, HID], f16, "w1r")
            Wzo_sb = load(Wzo_d, [P, 8], f16, "wzo")
            b1_sb = load(b1p_d, [P, 2], f32, "b1")
            b2_sb = load(b2b_d, [P, 2], f32, "b2")
            recip_sb = load(recip_d, [P, NB], f32, "recip")

            # replicate the 16-partition index block across all 8 core groups
            idx_sb = big.tile([P, NB * 8 * W2], i16, tag="idx", name="idx_sb")
            for g in range(8):
                nc.sync.dma_start(
                    out=idx_sb[16 * g:16 * (g + 1), :], in_=idx_d[:, :]
                )

            # iota / identity built on device
            ioti = big.tile([P, P], i32, tag="ioti", name="ioti")
            nc.gpsimd.iota(out=ioti[:], pattern=[[1, P]], base=0,
                           channel_multiplier=0)
            iotp = big.tile([P, P], i32, tag="iotp", name="iotp")
            nc.gpsimd.iota(out=iotp[:], pattern=[[0, P]], base=0,
                           channel_multiplier=1)
            iota_sb = big.tile([P, P], f32, tag="iota", name="iota_sb")
            nc.vector.tensor_copy(out=iota_sb[:], in_=ioti[:])
            identh = big.tile([P, P], f16, tag="identh", name="identh")
            nc.vector.tensor_tensor(
                out=identh[:], in0=ioti[:], in1=iotp[:],
                op=mybir.AluOpType.is_equal,
            )

            # x shard -> internal DRAM -> AllGather to full x
            x_int = dp.tile([ROWS, IN_C], f16, tag="xint", name="x_int")
            nc.sync.dma_start(out=x_int[:, :], in_=x_sh_d[:, :])
            x_full = dp.tile([NP, IN_C], f16, tag="xfull", name="x_full",
                             addr_space="Shared")
            nc.gpsimd.collective_compute(
                "AllGather",
                mybir.AluOpType.bypass,
                replica_groups=[list(range(NCORES))],
                ins=[x_int[:, :]],
                outs=[x_full[:, :]],
            )
            xb_src = x_int[:, :].rearrange("(p b) c -> p b c", b=NB)

            # layer-2 z table: 256B-padded rows, node order
            z_own = dp.tile([ROWS, P], f16, tag="zown", name="z_own")
            z_own_v = z_own[:, :].rearrange("(p b) f -> p b f", b=NB)
            z_full = dp.tile([NP, P], f16, tag="zfull", name="z_full",
                             addr_space="Shared")
            o_stage = dp.tile([P, 2 * NB], f32, tag="ostage", name="o_stage")

            out_sb = big.tile([P, 2 * NB], f32, tag="outs", name="out_sb")

            with tc.For_i(0, NB, name="l1") as b:
                g0 = lp.tile([P, W0, IN_C], f16, tag="g0", name="g0")
                nc.gpsimd.dma_gather(
                    out_ap=g0[:, :, :],
                    in_ap=x_full[0:HALF, :],
                    idxs_ap=idx_sb[:, ds(b * 8 * W2, 8 * W0)],
                    num_idxs=W0 * P,
                    num_idxs_reg=W0 * P,
                    elem_size=IN_C,
                    single_packet=False,
                )
                g1 = lp.tile([P, W1, IN_C], f16, tag="g1", name="g1")
                nc.gpsimd.dma_gather(
                    out_ap=g1[:, :, :],
                    in_ap=x_full[HALF:NP, :],
                    idxs_ap=idx_sb[:, ds(b * 8 * W2 + 8 * W0, 8 * W1)],
                    num_idxs=W1 * P,
                    num_idxs_reg=W1 * P,
                    elem_size=IN_C,
                    single_packet=False,
                )
                ldb_u = lp.tile([P, W2], u8, tag="ldbu", name="ldb_u")
                nc.sync.dma_start(out=ldb_u[:], in_=ldstu_d[:, ds(b * W2, W2)])
                ldb = lp.tile([P, W2], f32, tag="ldb", name="ldb")
                nc.vector.tensor_copy(out=ldb[:], in_=ldb_u[:])

                pagg = pp.tile([P, P], f32, tag="agg", name="pagg")
                for k in range(W2):
                    Pt = lp.tile([P, P], f16, tag="P", name="Pt")
                    nc.vector.tensor_scalar(
                        out=Pt[:], in0=iota_sb[:],
                        scalar1=ldb[:, k:k + 1], scalar2=None,
                        op0=mybir.AluOpType.is_equal,
                    )
                    rhs = g0[:, k, :] if k < W0 else g1[:, k - W0, :]
                    nc.tensor.matmul(
                        out=pagg[:], lhsT=Pt[:], rhs=rhs,
                        start=(k == 0), stop=(k == W2 - 1),
                    )
                rcb = lp.tile([P, 1], f32, tag="rcb", name="rcb")
                nc.sync.dma_start(out=rcb[:], in_=recip_d[:, ds(b, 1)])
                aggm = lp.tile([P, P], f16, tag="aggm", name="aggm")
                nc.vector.tensor_scalar(
                    out=aggm[:], in0=pagg[:], scalar1=rcb[:, 0:1],
                    scalar2=None, op0=mybir.AluOpType.mult,
                )
                ptr = pp.tile([P, P], f16, tag="tr", name="ptr", bufs=3)
                nc.tensor.transpose(out=ptr[:], in_=aggm[:], identity=identh[:])
                aggmT = lp.tile([P, P], f16, tag="aggmT", name="aggmT")
                nc.vector.tensor_copy(out=aggmT[:], in_=ptr[:])

                xb = lp.tile([P, IN_C], f16, tag="xb", name="xb")
                nc.sync.dma_start(out=xb[:], in_=xb_src[:, ds(b, 1), :])
                ptr2 = pp.tile([P, P], f16, tag="tr", name="ptr2", bufs=3)
                nc.tensor.transpose(out=ptr2[:], in_=xb[:], identity=identh[:])
                xbT = lp.tile([P, P], f16, tag="xbT", name="xbT")
                nc.vector.tensor_copy(out=xbT[:], in_=ptr2[:])

                hbT = []
                for j in range(2):
                    ph = pp.tile([P, P], f32, tag="tr", name="ph", bufs=3)
                    nc.tensor.matmul(
                        out=ph[:], lhsT=W1lT_sb[:, j * P:(j + 1) * P],
                        rhs=aggmT[:], start=True, stop=False,
                    )
                    nc.tensor.matmul(
                        out=ph[:], lhsT=W1rT_sb[:, j * P:(j + 1) * P],
                        rhs=xbT[:], start=False, stop=True,
                    )
                    ht = lp.tile([P, P], f16, tag=f"hbT{j}", name=f"ht{j}")
                    nc.scalar.activation(
                        out=ht[:], in_=ph[:],
                        func=mybir.ActivationFunctionType.Relu,
                        bias=b1_sb[:, j:j + 1],
                    )
                    hbT.append(ht)
                pzo = pp.tile([P, 4], f32, tag="zo", name="pzo", bufs=1)
                for j in range(2):
                    nc.tensor.matmul(
                        out=pzo[:], lhsT=hbT[j][:],
                        rhs=Wzo_sb[:, 4 * j:4 * j + 4],
                        start=(j == 0), stop=(j == 1),
                    )
                zb = lp.tile([P, 2], f16, tag="zb", name="zb")
                nc.vector.tensor_copy(out=zb[:], in_=pzo[:, 0:2])
                nc.sync.dma_start(out=z_own_v[:, ds(b, 1), 0:2], in_=zb[:])
                ob = lp.tile([P, 2], f32, tag="ob", name="ob")
                nc.vector.tensor_tensor(
                    out=ob[:], in0=pzo[:, 2:4], in1=b2_sb[:],
                    op=mybir.AluOpType.add,
                )
                nc.sync.dma_start(out=o_stage[:, ts(b, 2)], in_=ob[:])

            nc.gpsimd.collective_compute(
                "AllGather",
                mybir.AluOpType.bypass,
                replica_groups=[list(range(NCORES))],
                ins=[z_own[:, :]],
                outs=[z_full[:, :]],
            )

            with tc.For_i(0, NB, name="l2") as b:
                zg0 = lp.tile([P, W0, P], f16, tag="zg0", name="zg0")
                nc.gpsimd.dma_gather(
                    out_ap=zg0[:, :, :],
                    in_ap=z_full[0:HALF, :],
                    idxs_ap=idx_sb[:, ds(b * 8 * W2, 8 * W0)],
                    num_idxs=W0 * P,
                    num_idxs_reg=W0 * P,
                    elem_size=P,
                    single_packet=False,
                )
                zg1 = lp.tile([P, W1, P], f16, tag="zg1", name="zg1")
                nc.gpsimd.dma_gather(
                    out_ap=zg1[:, :, :],
                    in_ap=z_full[HALF:NP, :],
                    idxs_ap=idx_sb[:, ds(b * 8 * W2 + 8 * W0, 8 * W1)],
                    num_idxs=W1 * P,
                    num_idxs_reg=W1 * P,
                    elem_size=P,
                    single_packet=False,
                )
                ldb_u = lp.tile([P, W2], u8, tag="ldbu", name="ldb_u2")
                nc.sync.dma_start(out=ldb_u[:], in_=ldstu_d[:, ds(b * W2, W2)])
                ldb = lp.tile([P, W2], f32, tag="ldb", name="ldb2")
                nc.vector.tensor_copy(out=ldb[:], in_=ldb_u[:])

                pa2 = pp.tile([P, 2], f32, tag="agg2", name="pa2")
                for k in range(W2):
                    P2 = lp.tile([P, P], f16, tag="P", name="P2")
                    nc.vector.tensor_scalar(
                        out=P2[:], in0=iota_sb[:],
                        scalar1=ldb[:, k:k + 1], scalar2=None,
                        op0=mybir.AluOpType.is_equal,
                    )
                    rhs = (zg0[:, k, 0:2] if k < W0 else zg1[:, k - W0, 0:2])
                    nc.tensor.matmul(
                        out=pa2[:], lhsT=P2[:], rhs=rhs,
                        start=(k == 0), stop=(k == W2 - 1),
                    )
                rcb = lp.tile([P, 1], f32, tag="rcb", name="rcb2")
                nc.sync.dma_start(out=rcb[:], in_=recip_d[:, ds(b, 1)])
                red2 = lp.tile([P, 2], f32, tag="red2", name="red2")
                nc.vector.tensor_scalar(
                    out=red2[:], in0=pa2[:], scalar1=rcb[:, 0:1],
                    scalar2=None, op0=mybir.AluOpType.mult,
                )
                ob = lp.tile([P, 2], f32, tag="ob", name="ob2")
                nc.sync.dma_start(out=ob[:], in_=o_stage[:, ts(b, 2)])
                outb = lp.tile([P, 2], f32, tag="outb", name="outb")
                nc.vector.tensor_tensor(
                    out=outb[:], in0=red2[:], in1=ob[:],
                    op=mybir.AluOpType.add,
                )
                nc.vector.tensor_copy(out=out_sb[:, ts(b, 2)], in_=outb[:])

            nc.sync.dma_start(out=out_d[:, :], in_=out_sb[:])
    nc.compile()
    return nc


def _run(inputs, repeat=1):
    in_maps, W0, W1 = _host_prep(**inputs)
    nc = _build(W0, W1)
    best = None
    for _ in range(repeat):
        t0 = time.perf_counter()
        res = run_bass_kernel_spmd(
            nc, [dict(m) for m in in_maps], core_ids=list(range(NCORES))
        )
        dt = time.perf_counter() - t0
        print(f"  spmd run: {dt:.3f}s", flush=True)
        best = dt if best is None else min(best, dt)
    outs = []
    for c in range(NCORES):
        a = res.results[c]["out"]  # [128, 98]; row p, col 2b+f = node p*49+b
        outs.append(a.reshape(ROWS, 2))
    full = np.concatenate(outs, axis=0)[:N]
    return full.astype(np.float32), best


def kernel(**inputs):
    out, _ = _run(inputs, repeat=1)
    return out


# revision 10
# speedup vs baseline: 1.2338x; 1.2338x over previous
"""GraphSAGE 2-layer fraud detector on 8 trn2 NeuronCores.

Strategy (dst-partitioned, matmul scatter, minimal host->device traffic):
  - The axon tunnel moves ~45MB/s, so wall time is dominated by (a) input
    bytes shipped per spmd call and (b) per-call re-lowering of the kernel
    BIR (proportional to instruction count). Each core receives ONLY its
    x shard, quantized to int8 with a per-node fp16 scale (0.8MB), plus
    compressed edge tables; x is AllGathered across cores on-device, and
    everything else (iota, identity, x^T blocks, the z table) is derived
    on-device. All loops are tc.For_i hardware loops, so the kernel is a
    few hundred instructions regardless of edge count.
  - Nodes padded to 50176 = 8 cores x 49 blocks x 128. Core c owns nodes
    [c*6272, (c+1)*6272). Within a core, dst block b holds the 128 nodes
    with local index p*49 + b (p = row in block), which makes the z tile a
    plain contiguous view of z rows in node order.
  - Per-edge work is driven by gpsimd.dma_gather: one instruction gathers
    a whole block's 256B rows from an HBM table into SBUF. Rows pack two
    consecutive nodes (int8 x: 2x128B; z: 2x2 fp16 values in a padded
    row), so indices are src>>1 and fit int16. The parity selection AND
    the int8 dequant scale are folded into the one-hot scatter matrices:
      agg = sum_k [(iota==ldst_k)*sclE_k].T @ q_even
                + [(iota==ldst_k)*sclO_k].T @ q_odd
    where sclE/sclO = scale[src] masked by src parity (one fused
    tensor_scalar builds each matrix). Layer 2 uses the SAME index/ldst
    tables with parity masks instead of scales.
  - z = h@W2l.T, o = h@W2r.T + b2 (aggregation commutes with the linear
    map, so layer 2 aggregates 2-wide z, not 256-wide h); out =
    recip*agg2 + o.
"""

import time

import numpy as np

import concourse.bass as bass
import concourse.mybir as mybir
import concourse.tile as tile
from concourse import bacc
from concourse.bass import ds, ts
from concourse.bass_utils import run_bass_kernel_spmd

N = 50000
E = 800000
IN_C = 128
HID = 256
OUT_C = 2
NCORES = 8
P = 128
NB = 49                 # dst blocks per core
ROWS = NB * P           # 6272 rows per core
NP = NCORES * ROWS      # 50176 padded nodes
HNP = NP // 2           # 25088 paired rows (int16-addressable)

f32 = mybir.dt.float32
f16 = mybir.dt.float16
i32 = mybir.dt.int32
i16 = mybir.dt.int16
i8 = mybir.dt.int8
u8 = mybir.dt.uint8


def _wrap16(flat):
    """dma_gather index layout: flat j -> [partition j%16, col j//16]."""
    return np.ascontiguousarray(flat.reshape(-1, 16).T)


def _host_prep(x, edge_index, W1l, b1, W1r, W2l, b2, W2r):
    src = edge_index[0].astype(np.int64)
    dst = edge_index[1].astype(np.int64)
    cnt = np.bincount(dst, minlength=NP)
    recip = (1.0 / np.maximum(cnt, 1)).astype(np.float32)

    # int8 quantization of x with per-node fp16 scale
    x = np.asarray(x, np.float32)
    absmax = np.abs(x).max(axis=1)
    s_node = (np.maximum(absmax, 1e-6) / 127.0).astype(np.float16)
    s_full = np.ones(NP, np.float16)
    s_full[:N] = s_node
    q = np.zeros((NP, IN_C), np.int8)
    q[:N] = np.clip(np.rint(x / s_node.astype(np.float32)[:, None]),
                    -127, 127).astype(np.int8)

    # dst sort key in block-layout space: node (core c, local r) sits in
    # block b = r % 49 at row p = r // 49 -> key = c*6272 + b*128 + p.
    c_ = dst // ROWS
    r_ = dst % ROWS
    key = c_ * ROWS + (r_ % NB) * P + (r_ // NB)
    order = np.argsort(key, kind="stable")
    s_src = src[order]
    s_key = key[order]

    block_starts = np.searchsorted(s_key, np.arange(0, NP + P, P))
    cnt_blk = block_starts[1:] - block_starts[:-1]
    W = int(np.maximum(1, -(-cnt_blk // P)).max())  # uniform chunks per block
    C1 = NB * W

    idx_arr = np.zeros((NCORES, 16, NB * 8 * W), np.int16)
    ldst_arr = np.full((NCORES, P, C1), 255, dtype=np.uint8)
    par_arr = np.zeros((NCORES, P, C1), np.uint8)
    scl_arr = np.zeros((NCORES, P, C1), np.float16)
    for c in range(NCORES):
        for b in range(NB):
            bb = c * NB + b
            s, e = int(block_starts[bb]), int(block_starts[bb + 1])
            k = e - s
            bs = s_src[s:e]
            fi = np.zeros(W * P, np.int16)
            fi[:k] = bs >> 1
            idx_arr[c, :, b * 8 * W:(b + 1) * 8 * W] = _wrap16(fi)
            tl = np.full(W * P, 255, np.uint8)
            tl[:k] = (s_key[s:e] % P).astype(np.uint8)
            ldst_arr[c, :, b * W:(b + 1) * W] = tl.reshape(W, P).T
            tp = np.zeros(W * P, np.uint8)
            tp[:k] = (bs & 1).astype(np.uint8)
            par_arr[c, :, b * W:(b + 1) * W] = tp.reshape(W, P).T
            tsc = np.zeros(W * P, np.float16)
            tsc[:k] = s_full[bs]
            scl_arr[c, :, b * W:(b + 1) * W] = tsc.reshape(W, P).T

    W1lT = np.ascontiguousarray(W1l.T.astype(np.float16))   # [128, 256]
    W1rT = np.ascontiguousarray(W1r.T.astype(np.float16))
    Wzo = np.zeros((P, 8), np.float16)
    for j in range(2):
        Wzo[:, 4 * j:4 * j + 2] = W2l.T[j * P:(j + 1) * P, :].astype(np.float16)
        Wzo[:, 4 * j + 2:4 * j + 4] = W2r.T[j * P:(j + 1) * P, :].astype(np.float16)
    b1p = np.ascontiguousarray(np.asarray(b1).reshape(2, P).T.astype(np.float32))
    b2b = np.tile(np.asarray(b2).reshape(1, 2), (P, 1)).astype(np.float32)
    recip_c = recip.reshape(NCORES, P, NB).copy()   # node local r = p*49+b
    s_own = s_full.astype(np.float32).reshape(NCORES, P, NB)

    in_maps = []
    for c in range(NCORES):
        in_maps.append({
            "x_q": np.ascontiguousarray(
                q[c * ROWS:(c + 1) * ROWS, :].reshape(ROWS // 2, 2 * IN_C)),
            "idx16": np.ascontiguousarray(idx_arr[c]),
            "ldstu": np.ascontiguousarray(ldst_arr[c]),
            "paru": np.ascontiguousarray(par_arr[c]),
            "sclh": np.ascontiguousarray(scl_arr[c]),
            "sclown": np.ascontiguousarray(s_own[c]),
            "W1lT": W1lT,
            "W1rT": W1rT,
            "Wzo": Wzo,
            "b1p": b1p,
            "b2b": b2b,
            "recip": np.ascontiguousarray(recip_c[c]),
        })
    return in_maps, W


def _build(W):
    C1 = NB * W
    nc = bacc.Bacc(None, target_bir_lowering=False, debug=False)

    x_q_d = nc.dram_tensor("x_q", [ROWS // 2, 2 * IN_C], i8, kind="ExternalInput")
    idx_d = nc.dram_tensor("idx16", [16, NB * 8 * W], i16, kind="ExternalInput")
    ldstu_d = nc.dram_tensor("ldstu", [P, C1], u8, kind="ExternalInput")
    paru_d = nc.dram_tensor("paru", [P, C1], u8, kind="ExternalInput")
    sclh_d = nc.dram_tensor("sclh", [P, C1], f16, kind="ExternalInput")
    sclown_d = nc.dram_tensor("sclown", [P, NB], f32, kind="ExternalInput")
    W1lT_d = nc.dram_tensor("W1lT", [P, HID], f16, kind="ExternalInput")
    W1rT_d = nc.dram_tensor("W1rT", [P, HID], f16, kind="ExternalInput")
    Wzo_d = nc.dram_tensor("Wzo", [P, 8], f16, kind="ExternalInput")
    b1p_d = nc.dram_tensor("b1p", [P, 2], f32, kind="ExternalInput")
    b2b_d = nc.dram_tensor("b2b", [P, 2], f32, kind="ExternalInput")
    recip_d = nc.dram_tensor("recip", [P, NB], f32, kind="ExternalInput")
    out_d = nc.dram_tensor("out", [P, 2 * NB], f32, kind="ExternalOutput")

    with tile.TileContext(nc) as tc:
        with (
            tc.tile_pool(name="big", bufs=1) as big,
            tc.tile_pool(name="lp", bufs=4) as lp,
            tc.tile_pool(name="pp", bufs=2, space="PSUM") as pp,
            tc.tile_pool(name="dram", bufs=1, space="DRAM") as dp,
        ):
            def load(d, shape, dt, tag):
                t = big.tile(shape, dt, tag=tag, name=tag)
                nc.sync.dma_start(out=t[:], in_=d[:, :])
                return t

            W1lT_sb = load(W1lT_d, [P, HID], f16, "w1l")
            W1rT_sb = load(W1rT_d, [P, HID], f16, "w1r")
            Wzo_sb = load(Wzo_d, [P, 8], f16, "wzo")
            b1_sb = load(b1p_d, [P, 2], f32, "b1")
            b2_sb = load(b2b_d, [P, 2], f32, "b2")
            ldstu_sb = load(ldstu_d, [P, C1], u8, "ldstu")
            paru_sb = load(paru_d, [P, C1], u8, "paru")
            sclh_sb = load(sclh_d, [P, C1], f16, "sclh")

            # replicate the 16-partition index block across all 8 core groups
            idx_sb = big.tile([P, NB * 8 * W], i16, tag="idx", name="idx_sb")
            for g in range(8):
                nc.sync.dma_start(
                    out=idx_sb[16 * g:16 * (g + 1), :], in_=idx_d[:, :]
                )

            # widened tables: ldst f32; parity/scale masks (f32 scalars)
            ldst_sb = big.tile([P, C1], f32, tag="ldst", name="ldst_sb")
            nc.vector.tensor_copy(out=ldst_sb[:], in_=ldstu_sb[:])
            parO = big.tile([P, C1], f32, tag="parO", name="parO")
            nc.vector.tensor_copy(out=parO[:], in_=paru_sb[:])
            parE = big.tile([P, C1], f32, tag="parE", name="parE")
            nc.vector.tensor_scalar(
                out=parE[:], in0=parO[:], scalar1=-1.0, scalar2=1.0,
                op0=mybir.AluOpType.mult, op1=mybir.AluOpType.add,
            )
            scl = big.tile([P, C1], f32, tag="scl", name="scl")
            nc.vector.tensor_copy(out=scl[:], in_=sclh_sb[:])
            sclE = big.tile([P, C1], f32, tag="sclE", name="sclE")
            nc.vector.tensor_tensor(
                out=sclE[:], in0=scl[:], in1=parE[:], op=mybir.AluOpType.mult)
            sclO = big.tile([P, C1], f32, tag="sclO", name="sclO")
            nc.vector.tensor_tensor(
                out=sclO[:], in0=scl[:], in1=parO[:], op=mybir.AluOpType.mult)

            # iota / identity built on device
            ioti = big.tile([P, P], i32, tag="ioti", name="ioti")
            nc.gpsimd.iota(out=ioti[:], pattern=[[1, P]], base=0,
                           channel_multiplier=0)
            iotp = big.tile([P, P], i32, tag="iotp", name="iotp")
            nc.gpsimd.iota(out=iotp[:], pattern=[[0, P]], base=0,
                           channel_multiplier=1)
            iota_sb = big.tile([P, P], f32, tag="iota", name="iota_sb")
            nc.vector.tensor_copy(out=iota_sb[:], in_=ioti[:])
            identh = big.tile([P, P], f16, tag="identh", name="identh")
            nc.vector.tensor_tensor(
                out=identh[:], in0=ioti[:], in1=iotp[:],
                op=mybir.AluOpType.is_equal,
            )

            # x (int8, two nodes per 256B row) -> internal DRAM -> AllGather
            x_int = dp.tile([ROWS // 2, 2 * IN_C], i8, tag="xint", name="x_int")
            nc.sync.dma_start(out=x_int[:, :], in_=x_q_d[:, :])
            x_full = dp.tile([HNP, 2 * IN_C], i8, tag="xfull", name="x_full",
                             addr_space="Shared")
            nc.gpsimd.collective_compute(
                "AllGather",
                mybir.AluOpType.bypass,
                replica_groups=[list(range(NCORES))],
                ins=[x_int[:, :]],
                outs=[x_full[:, :]],
            )
            xb_src = (x_int[:, :]
                      .rearrange("g (t c) -> (g t) c", t=2)
                      .rearrange("(p b) c -> p b c", b=NB))

            z_own = dp.tile([ROWS, 2], f16, tag="zown", name="z_own")
            z_own_v = z_own[:, :].rearrange("(p b) f -> p b f", b=NB)
            z_all = dp.tile([NP, 2], f16, tag="zall", name="z_all",
                            addr_space="Shared")
            z2 = dp.tile([HNP, P], f16, tag="z2", name="z2")
            o_stage = dp.tile([P, 2 * NB], f32, tag="ostage", name="o_stage")

            out_sb = big.tile([P, 2 * NB], f32, tag="outs", name="out_sb")

            with tc.For_i(0, NB, name="l1") as b:
                g = lp.tile([P, W, 2 * IN_C], i8, tag="g", name="g")
                nc.gpsimd.dma_gather(
                    out_ap=g[:, :, :],
                    in_ap=x_full[:, :],
                    idxs_ap=idx_sb[:, ds(b * 8 * W, 8 * W)],
                    num_idxs=W * P,
                    num_idxs_reg=W * P,
                    elem_size=2 * IN_C,
                    single_packet=False,
                )
                gf = lp.tile([P, W, 2 * IN_C], f16, tag="gf", name="gf")
                nc.vector.tensor_copy(out=gf[:, :, :], in_=g[:, :, :])
                ldb_u = lp.tile([P, W], u8, tag="ldbu", name="ldb_u")
                nc.sync.dma_start(out=ldb_u[:], in_=ldstu_d[:, ds(b * W, W)])
                ldb = lp.tile([P, W], f32, tag="ldb", name="ldb")
                nc.vector.tensor_copy(out=ldb[:], in_=ldb_u[:])

                pagg = pp.tile([P, P], f32, tag="agg", name="pagg")
                for k in range(W):
                    PtE = lp.tile([P, P], f16, tag="P", name="PtE")
                    nc.vector.tensor_scalar(
                        out=PtE[:], in0=iota_sb[:],
                        scalar1=ldb[:, k:k + 1], scalar2=sclE[:, ds(b * W + k, 1)],
                        op0=mybir.AluOpType.is_equal, op1=mybir.AluOpType.mult,
                    )
                    nc.tensor.matmul(
                        out=pagg[:], lhsT=PtE[:], rhs=gf[:, k, 0:IN_C],
                        start=(k == 0), stop=False,
                    )
                    PtO = lp.tile([P, P], f16, tag="P", name="PtO")
                    nc.vector.tensor_scalar(
                        out=PtO[:], in0=iota_sb[:],
                        scalar1=ldb[:, k:k + 1], scalar2=sclO[:, ds(b * W + k, 1)],
                        op0=mybir.AluOpType.is_equal, op1=mybir.AluOpType.mult,
                    )
                    nc.tensor.matmul(
                        out=pagg[:], lhsT=PtO[:], rhs=gf[:, k, IN_C:2 * IN_C],
                        start=False, stop=(k == W - 1),
                    )
                rcb = lp.tile([P, 1], f32, tag="rcb", name="rcb")
                nc.sync.dma_start(out=rcb[:], in_=recip_d[:, ds(b, 1)])
                aggm = lp.tile([P, P], f16, tag="aggm", name="aggm")
                nc.vector.tensor_scalar(
                    out=aggm[:], in0=pagg[:], scalar1=rcb[:, 0:1],
                    scalar2=None, op0=mybir.AluOpType.mult,
                )
                ptr = pp.tile([P, P], f16, tag="tr", name="ptr", bufs=3)
                nc.tensor.transpose(out=ptr[:], in_=aggm[:], identity=identh[:])
                aggmT = lp.tile([P, P], f16, tag="aggmT", name="aggmT")
                nc.vector.tensor_copy(out=aggmT[:], in_=ptr[:])

                xb_q = lp.tile([P, IN_C], i8, tag="xbq", name="xb_q")
                nc.sync.dma_start(out=xb_q[:], in_=xb_src[:, ds(b, 1), :])
                scob = lp.tile([P, 1], f32, tag="scob", name="scob")
                nc.sync.dma_start(out=scob[:], in_=sclown_d[:, ds(b, 1)])
                xb = lp.tile([P, IN_C], f16, tag="xb", name="xb")
                nc.vector.tensor_scalar(
                    out=xb[:], in0=xb_q[:], scalar1=scob[:, 0:1],
                    scalar2=None, op0=mybir.AluOpType.mult,
                )
                ptr2 = pp.tile([P, P], f16, tag="tr", name="ptr2", bufs=3)
                nc.tensor.transpose(out=ptr2[:], in_=xb[:], identity=identh[:])
                xbT = lp.tile([P, P], f16, tag="xbT", name="xbT")
                nc.vector.tensor_copy(out=xbT[:], in_=ptr2[:])

                hbT = []
                for j in range(2):
                    ph = pp.tile([P, P], f32, tag="tr", name="ph", bufs=3)
                    nc.tensor.matmul(
                        out=ph[:], lhsT=W1lT_sb[:, j * P:(j + 1) * P],
                        rhs=aggmT[:], start=True, stop=False,
                    )
                    nc.tensor.matmul(
                        out=ph[:], lhsT=W1rT_sb[:, j * P:(j + 1) * P],
                        rhs=xbT[:], start=False, stop=True,
                    )
                    ht = lp.tile([P, P], f16, tag=f"hbT{j}", name=f"ht{j}")
                    nc.scalar.activation(
                        out=ht[:], in_=ph[:],
                        func=mybir.ActivationFunctionType.Relu,
                        bias=b1_sb[:, j:j + 1],
                    )
                    hbT.append(ht)
                pzo = pp.tile([P, 4], f32, tag="zo", name="pzo", bufs=1)
                for j in range(2):
                    nc.tensor.matmul(
                        out=pzo[:], lhsT=hbT[j][:],
                        rhs=Wzo_sb[:, 4 * j:4 * j + 4],
                        start=(j == 0), stop=(j == 1),
                    )
                zb = lp.tile([P, 2], f16, tag="zb", name="zb")
                nc.vector.tensor_copy(out=zb[:], in_=pzo[:, 0:2])
                nc.sync.dma_start(out=z_own_v[:, ds(b, 1), :], in_=zb[:])
                ob = lp.tile([P, 2], f32, tag="ob", name="ob")
                nc.vector.tensor_tensor(
                    out=ob[:], in0=pzo[:, 2:4], in1=b2_sb[:],
                    op=mybir.AluOpType.add,
                )
                nc.sync.dma_start(out=o_stage[:, ts(b, 2)], in_=ob[:])

            nc.gpsimd.collective_compute(
                "AllGather",
                mybir.AluOpType.bypass,
                replica_groups=[list(range(NCORES))],
                ins=[z_own[:, :]],
                outs=[z_all[:, :]],
            )
            # pack z pairs into 256B rows: z2[g, 0:4] = [z(2g) | z(2g+1)]
            nc.sync.dma_start(
                out=z2[:, 0:4],
                in_=z_all[:, :].rearrange("(g t) f -> g (t f)", t=2),
            )

            with tc.For_i(0, NB, name="l2") as b:
                zg = lp.tile([P, W, P], f16, tag="zg", name="zg")
                nc.gpsimd.dma_gather(
                    out_ap=zg[:, :, :],
                    in_ap=z2[:, :],
                    idxs_ap=idx_sb[:, ds(b * 8 * W, 8 * W)],
                    num_idxs=W * P,
                    num_idxs_reg=W * P,
                    elem_size=P,
                    single_packet=False,
                )
                ldb_u = lp.tile([P, W], u8, tag="ldbu", name="ldb_u2")
                nc.sync.dma_start(out=ldb_u[:], in_=ldstu_d[:, ds(b * W, W)])
                ldb = lp.tile([P, W], f32, tag="ldb", name="ldb2")
                nc.vector.tensor_copy(out=ldb[:], in_=ldb_u[:])

                pa2 = pp.tile([P, 2], f32, tag="agg2", name="pa2")
                for k in range(W):
                    P2E = lp.tile([P, P], f16, tag="P", name="P2E")
                    nc.vector.tensor_scalar(
                        out=P2E[:], in0=iota_sb[:],
                        scalar1=ldb[:, k:k + 1], scalar2=parE[:, ds(b * W + k, 1)],
                        op0=mybir.AluOpType.is_equal, op1=mybir.AluOpType.mult,
                    )
                    nc.tensor.matmul(
                        out=pa2[:], lhsT=P2E[:], rhs=zg[:, k, 0:2],
                        start=(k == 0), stop=False,
                    )
                    P2O = lp.tile([P, P], f16, tag="P", name="P2O")
                    nc.vector.tensor_scalar(
                        out=P2O[:], in0=iota_sb[:],
                        scalar1=ldb[:, k:k + 1], scalar2=parO[:, ds(b * W + k, 1)],
                        op0=mybir.AluOpType.is_equal, op1=mybir.AluOpType.mult,
                    )
                    nc.tensor.matmul(
                        out=pa2[:], lhsT=P2O[:], rhs=zg[:, k, 2:4],
                        start=False, stop=(k == W - 1),
                    )
                rcb = lp.tile([P, 1], f32, tag="rcb", name="rcb2")
                nc.sync.dma_start(out=rcb[:], in_=recip_d[:, ds(b, 1)])
                red2 = lp.tile([P, 2], f32, tag="red2", name="red2")
                nc.vector.tensor_scalar(
                    out=red2[:], in0=pa2[:], scalar1=rcb[:, 0:1],
                    scalar2=None, op0=mybir.AluOpType.mult,
                )
                ob = lp.tile([P, 2], f32, tag="ob", name="ob2")
                nc.sync.dma_start(out=ob[:], in_=o_stage[:, ts(b, 2)])
                outb = lp.tile([P, 2], f32, tag="outb", name="outb")
                nc.vector.tensor_tensor(
                    out=outb[:], in0=red2[:], in1=ob[:],
                    op=mybir.AluOpType.add,
                )
                nc.vector.tensor_copy(out=out_sb[:, ts(b, 2)], in_=outb[:])

            nc.sync.dma_start(out=out_d[:, :], in_=out_sb[:])
    nc.compile()
    return nc


def _run(inputs, repeat=1):
    in_maps, W = _host_prep(**inputs)
    nc = _build(W)
    best = None
    for _ in range(repeat):
        t0 = time.perf_counter()
        res = run_bass_kernel_spmd(
            nc, [dict(m) for m in in_maps], core_ids=list(range(NCORES))
        )
        dt = time.perf_counter() - t0
        print(f"  spmd run: {dt:.3f}s", flush=True)
        best = dt if best is None else min(best, dt)
    outs = []
    for c in range(NCORES):
        a = res.results[c]["out"]  # [128, 98]; row p, col 2b+f = node p*49+b
        outs.append(a.reshape(ROWS, 2))
    full = np.concatenate(outs, axis=0)[:N]
    return full.astype(np.float32), best


def kernel(**inputs):
    out, _ = _run(inputs, repeat=1)
    return out
